# revision 5
# baseline (speedup 1.0000x reference)
"""Trainium2 Bass kernel for nn_DotAtt_40097814675537 (v2: fp8 DoubleRow).

Math (matches the reference exactly up to fp rounding):
    score = Q @ K^T / sqrt(d)        [B, Sq, Sk]
    x     = score @ V                [B, Sq, dv]
    out   = softmax(where(j > valid_len[q], -1e6, x[b, q, j]), axis=-1)

Scheme:
  * Associativity: x = (Q / sqrt(d)) @ (K^T @ V)  (exact, 4x fewer FLOPs).
  * Data-parallel over batch B=8, one batch per NeuronCore.
  * Precision via fp16 main pass + fp8(e4m3) correction passes packed into
    ONE DoubleRow matmul (2 contraction rows/cycle = fp8 peak):
      a@b ~= ah16@bh16 + (a8 x blo8 + alo8 x b8)   [DoubleRow pair]
    All operands carry power-of-2 scales so every pass accumulates into the
    SAME psum bank (phase1 = 2^14*M, phase2 = 2^17*X); descales fold into
    the DVE reduce (x -2^-17) and ACT copies.  ~16.5 effective mantissa
    bits; simulated absmax 5.7e-4 / rel 9.7e-5 vs fp64 reference.
  * Sorted-query specialization: queries sorted by valid_len; per 128-row
    tile only columns [0, W) computed, W = roundup(max vl + 1, 32).  Tiles
    processed widest-first; outputs packed [128, sum_w] fp16, host divides
    by row sum (exact softmax normalization) and unpermutes.
  * Mask added (with the 2^-17 psum descale) in one fused DVE
    scalar_tensor_tensor per tile; row max via tensor_reduce(negate);
    exp via ACT with bias=-max, fp16 out.
  * Only the fp16 tensors + fp8 lo-residuals are DMA'd (~10MB/core); the
    coarse fp8 factors (k8/v8/q8) are derived on the idle vector engine
    from the fp16 data (tensor_scalar_mul -> e4m3), cutting DMA ~3MB/core.
  * DMA issues split across the two hardware DGE queues (sync + ACT) in
    fine 2-chunk blocks so semaphore granularity never stalls the PE;
    phase-1 fp16/fp8 passes grouped per s-chunk (DMA-paced), phase-2
    grouped per 4 tiles (fewer fp16<->fp8 weight-mode switches).  PE
    warmed up with dummy matmuls during the DMA head to ramp DVFS.
"""

import math
import sys
import types

import numpy as np

B, SQ, SK, D, DV = 8, 2048, 2048, 512, 512
N_CORES = 8
P = 128
SC = SK // P  # 16 s-chunks
DC = D // P  # 4 d-chunks
QT_TILES = SQ // P  # 16 query tiles
NEG_FILL = -1000000.0

_CACHE = {}


def _install_ntff_hook():
    if "antenv.axon_hooks" in sys.modules:
        return
    try:
        from trn_agent_boot.trn_boot import _ntff_profile_via_ctypes

        hook = _ntff_profile_via_ctypes("/opt/axon/libaxon_pjrt.so")
    except Exception:
        hook = None
    mod = types.ModuleType("antenv.axon_hooks")
    mod.get_axon_ntff_profile_hook = lambda: hook
    mod.set_axon_ntff_profile_hook = lambda h: None
    sys.modules["antenv.axon_hooks"] = mod


def _build(widths):
    """widths: per-tile column counts in PROCESSING order (widest first)."""
    import concourse.tile as tile
    from concourse import bacc, mybir

    nc = bacc.Bacc("TRN2", target_bir_lowering=False, debug=False, num_devices=N_CORES)
    f32 = mybir.dt.float32
    f16 = mybir.dt.float16
    bf16 = mybir.dt.bfloat16
    f8 = mybir.dt.float8e4
    Alu = mybir.AluOpType
    Act = mybir.ActivationFunctionType

    sum_w = sum(widths)
    offs = [0]
    for w in widths:
        offs.append(offs[-1] + w)
    # output DMA groups (last tile alone so the final transfer is tiny)
    gsplit = [(0, 4), (4, 8), (8, 12), (12, 15), (15, 16)]
    groups = [(offs[a], offs[b]) for a, b in gsplit]
    tile_group = {}
    for g, (a, b) in enumerate(gsplit):
        for t in range(a, b):
            tile_group[t] = g

    kh_d = nc.dram_tensor("kh", [P, SC, 512], f16, kind="ExternalInput")
    vh_d = nc.dram_tensor("vh", [P, SC, 512], f16, kind="ExternalInput")
    k8_d = nc.dram_tensor("k8p", [P, SC, 512], f8, kind="ExternalInput")
    v8_d = nc.dram_tensor("v8p", [P, SC, 512], f8, kind="ExternalInput")
    qt_d = nc.dram_tensor("qt", [P, QT_TILES, 512], f16, kind="ExternalInput")
    qt8_d = nc.dram_tensor("qt8p", [P, QT_TILES, 512], f8, kind="ExternalInput")
    mask_d = nc.dram_tensor("mask", [P, sum_w], bf16, kind="ExternalInput")
    o_d = nc.dram_tensor("o", [P, sum_w], f16, kind="ExternalOutput")

    with tile.TileContext(nc) as tc:
        with (
            tc.tile_pool(name="big", bufs=1) as big,
            tc.tile_pool(name="mtiles", bufs=1) as mtiles,
            tc.tile_pool(name="tmp", bufs=2) as tmppool,
            tc.tile_pool(name="work", bufs=4) as work,
            tc.tile_pool(name="stats", bufs=8) as stats,
            tc.tile_pool(name="psm", bufs=1, space="PSUM") as psm,
            tc.tile_pool(name="psx", bufs=4, space="PSUM") as psx,
        ):
            kh = big.tile([P, SC, 512], f16, tag="kh", name="kh_sb")
            vh = big.tile([P, SC, 512], f16, tag="vh", name="vh_sb")
            k8p = big.tile([P, SC, 2, 512], f8, tag="k8p", name="k8p_sb")
            v8p = big.tile([P, SC, 2, 512], f8, tag="v8p", name="v8p_sb")
            qt = big.tile([P, QT_TILES, 512], f16, tag="qt", name="qt_sb")
            qt8p = big.tile([P, QT_TILES, 2, 512], f8, tag="qt8p", name="qt8p_sb")
            mask_t = big.tile([P, sum_w], bf16, tag="mask", name="mask_sb")
            og = [
                big.tile([P, g1 - g0], f16, tag=f"og{g}", name=f"og{g}")
                for g, (g0, g1) in enumerate(groups)
            ]

            # warmup tiles (zeros) to ramp the PE clock during the DMA head
            wk = big.tile([P, P], f16, tag="wk", name="wk")
            wm = big.tile([P, 64], f16, tag="wm", name="wm")
            nc.vector.memset(wk[:, :], 0)
            nc.vector.memset(wm[:, :], 0)

            # ---- DMA issue: 2 HW queues, phase-1 first, fine blocks ----
            kblocks = [(0, 1), (1, 2), (2, 4), (4, 6), (6, 8), (8, 10),
                       (10, 12), (12, 14), (14, 16)]
            for s0, s1 in kblocks:
                nc.sync.dma_start(out=kh[:, s0:s1, :], in_=kh_d[:, s0:s1, :])
                nc.scalar.dma_start(out=vh[:, s0:s1, :], in_=vh_d[:, s0:s1, :])
                nc.sync.dma_start(out=k8p[:, s0:s1, 1, :], in_=k8_d[:, s0:s1, :])
                nc.scalar.dma_start(out=v8p[:, s0:s1, 0, :], in_=v8_d[:, s0:s1, :])
                # hi-derived fp8 factors cast on the idle vector engine
                nc.vector.tensor_scalar_mul(
                    k8p[:, s0:s1, 0, :], kh[:, s0:s1, :], 2.0**-3
                )
                nc.vector.tensor_scalar_mul(
                    v8p[:, s0:s1, 1, :], vh[:, s0:s1, :], 2.0**-3
                )
            half = offs[8]
            nc.scalar.dma_start(out=mask_t[:, 0:half], in_=mask_d[:, 0:half])
            nc.scalar.dma_start(out=mask_t[:, half:sum_w], in_=mask_d[:, half:sum_w])
            qblocks = [(0, 4), (4, 8), (8, 12), (12, 16)]
            for t0, t1 in qblocks:
                nc.sync.dma_start(out=qt[:, t0:t1, :], in_=qt_d[:, t0:t1, :])
                nc.scalar.dma_start(
                    out=qt8p[:, t0:t1, 1, :], in_=qt8_d[:, t0:t1, :]
                )
            for t0, t1 in qblocks[:2]:
                nc.vector.tensor_scalar_mul(
                    qt8p[:, t0:t1, 0, :], qt[:, t0:t1, :], 2.0**-8
                )

            # ---- warmup matmuls (PE p-state ramp during DMA head) ----
            for i in range(20):
                pw = psx.tile([P, 512], f32, tag="x", name="pwarm")
                nc.tensor.matmul(
                    pw[:, 0:64], wk[:, :], wm[:, :], start=True, stop=True
                )

            # ---- phase 1: psum_c = 2^14 * M_c, interleaved hh/DR per s ----
            pm = [
                psm.tile([P, 512], f32, tag=f"m{c}", name=f"pm{c}") for c in range(DC)
            ]
            for s in range(SC):
                for c in range(DC):
                    nc.tensor.matmul(
                        pm[c][:, :],
                        kh[:, s, c * P : (c + 1) * P],
                        vh[:, s, :],
                        start=(s == 0),
                        stop=False,
                    )
                for c in range(DC):
                    nc.tensor.matmul(
                        pm[c][:, :],
                        k8p[:, s, :, c * P : (c + 1) * P],
                        v8p[:, s, :, :],
                        start=False,
                        stop=(s == SC - 1),
                        perf_mode=mybir.MatmulPerfMode.DoubleRow,
                    )

            # ---- M readout: Mhi16 (ACT), 2^14*Mlo (DVE stt), Mlo8 (ACT),
            #      M8 (GPSIMD) ----
            mhi = [
                mtiles.tile([P, 512], f16, tag=f"mh{c}", name=f"mhi{c}")
                for c in range(DC)
            ]
            mp8 = [
                mtiles.tile([P, 2, 512], f8, tag=f"mp{c}", name=f"mp8{c}")
                for c in range(DC)
            ]
            for c in range(DC):
                nc.scalar.activation(
                    mhi[c][:, :], pm[c][:, :], Act.Copy, bias=0.0, scale=2.0**-14
                )
            tmps = []
            for c in range(DC):
                t32 = tmppool.tile([P, 512], f32, tag="t32", name=f"t32_{c}")
                nc.vector.scalar_tensor_tensor(
                    t32[:, :], mhi[c][:, :], -(2.0**14), pm[c][:, :],
                    op0=Alu.mult, op1=Alu.add,
                )
                nc.vector.tensor_scalar_mul(mp8[c][:, 1, :], mhi[c][:, :], 2.0**-3)
                tmps.append(t32)
            for c in range(DC):
                nc.scalar.activation(
                    mp8[c][:, 0, :], tmps[c][:, :], Act.Copy, bias=0.0, scale=2.0**-6
                )
            for t0, t1 in [(8, 12), (12, 16)]:
                nc.vector.tensor_scalar_mul(
                    qt8p[:, t0:t1, 0, :], qt[:, t0:t1, :], 2.0**-8
                )

            # ---- phase 2 per tile: psum = 2^17 * X, then fused mask+max,
            #      exp to fp16 packed output ----
            ogq = {0: "sync", 1: "scalar", 2: "sync", 3: "scalar", 4: "sync"}
            for t4 in range(0, QT_TILES, 4):
                grp = range(t4, min(t4 + 4, QT_TILES))
                pxs = {}
                for t in grp:
                    W = widths[t]
                    px = psx.tile([P, 512], f32, tag="x", name=f"px{t}")
                    pxs[t] = px
                    for c in range(DC):
                        nc.tensor.matmul(
                            px[:, 0:W],
                            qt[:, t, c * P : (c + 1) * P],
                            mhi[c][:, 0:W],
                            start=(c == 0),
                            stop=False,
                        )
                for t in grp:
                    W = widths[t]
                    px = pxs[t]
                    for c in range(DC):
                        nc.tensor.matmul(
                            px[:, 0:W],
                            qt8p[:, t, :, c * P : (c + 1) * P],
                            mp8[c][:, :, 0:W],
                            start=False,
                            stop=(c == DC - 1),
                            perf_mode=mybir.MatmulPerfMode.DoubleRow,
                        )
                for t in grp:
                    W = widths[t]
                    px = pxs[t]
                    xs = work.tile([P, 512], f32, tag="xn", name=f"xs{t}")
                    nmx = stats.tile([P, 1], f32, tag="nmx", name=f"nmx{t}")
                    nc.vector.scalar_tensor_tensor(
                        xs[:, 0:W], px[:, 0:W], 2.0**-17,
                        mask_t[:, offs[t] : offs[t] + W],
                        op0=Alu.mult, op1=Alu.add,
                    )
                    nc.vector.tensor_reduce(
                        out=nmx,
                        in_=xs[:, 0:W],
                        axis=mybir.AxisListType.X,
                        op=Alu.max,
                        negate=True,
                    )
                    g = tile_group[t]
                    g0, _ = groups[g]
                    nc.scalar.activation(
                        og[g][:, offs[t] - g0 : offs[t] - g0 + W],
                        xs[:, 0:W],
                        Act.Exp,
                        bias=nmx[:, :],
                        scale=1.0,
                    )
                    if t in (3, 7, 11, 14, 15):
                        g0, g1 = groups[g]
                        eng = nc.sync if ogq[g] == "sync" else nc.scalar
                        eng.dma_start(out=o_d[:, g0:g1], in_=og[g][:, :])

    nc.compile()
    return nc


def _get_nc(widths):
    key = tuple(widths)
    if key not in _CACHE:
        _install_ntff_hook()
        _CACHE[key] = _build(key)
    return _CACHE[key]


def _part_major(x, cols):
    """[G*128, cols] -> [128, G, cols]."""
    g = x.shape[0] // P
    return np.ascontiguousarray(x.reshape(g, P, cols).transpose(1, 0, 2))


def kernel(K, V, Q, valid_len, _trace=False):
    import ml_dtypes

    from concourse.bass_utils import run_bass_kernel_spmd

    E4 = ml_dtypes.float8_e4m3fn
    BF = ml_dtypes.bfloat16

    K = np.ascontiguousarray(np.asarray(K, dtype=np.float32))
    V = np.ascontiguousarray(np.asarray(V, dtype=np.float32))
    Q = np.asarray(Q, dtype=np.float32)
    vl = np.asarray(valid_len).astype(np.int64)

    # sort queries by valid_len; width per sorted tile, processing order =
    # widest first
    perm = np.argsort(vl, kind="stable")
    vls = vl[perm]
    widths_sorted = []
    for t in range(QT_TILES):
        w = int(vls[t * P : (t + 1) * P].max()) + 1
        widths_sorted.append(min(DV, -(-w // 32) * 32))
    order = sorted(
        range(QT_TILES), key=lambda i: widths_sorted[i], reverse=True
    )
    widths = tuple(widths_sorted[t] for t in order)
    sum_w = sum(widths)
    offs = [0]
    for w in widths:
        offs.append(offs[-1] + w)

    # ---- phase-1 operands ----
    # kh = f16(K*2^7); klo = K - kh*2^-7; k8 = e4(K*2^4); klo8 = e4(klo*2^10)
    def p1_pack(X):
        xh = (X * np.float32(2.0**7)).astype(np.float16)
        xlo = X - xh.astype(np.float32) * np.float32(2.0**-7)
        xlo8 = np.clip(xlo * np.float32(2.0**10), -440, 440).astype(E4)
        return xh, xlo8

    scale = np.float32(1.0 / math.sqrt(D))

    # mask (2^17-scaled additive), packed in processing order
    col = np.arange(DV, dtype=np.int64)
    mask_full = np.where(
        col[None, :] > vls[:, None], np.float32(NEG_FILL), np.float32(0.0)
    )
    mask_packed = np.empty((P, sum_w), dtype=BF)
    for ti, t in enumerate(order):
        w = widths[ti]
        mask_packed[:, offs[ti] : offs[ti] + w] = mask_full[
            t * P : (t + 1) * P, :w
        ].astype(BF)

    nc = _get_nc(widths)

    in_maps = []
    for b in range(N_CORES):
        khh, klo8 = p1_pack(K[b])
        vhh, vlo8 = p1_pack(V[b])
        kh_pm = _part_major(khh, 512)
        vh_pm = _part_major(vhh, 512)
        # DR pairs: (k8 x vlo8) + (klo8 x v8); k8/v8 are derived on-device
        k8p = _part_major(klo8, 512)
        v8p = _part_major(vlo8, 512)

        qs = Q[b][perm] * scale  # [SQ, D]
        qh = (qs * np.float32(2.0**17)).astype(np.float16)
        qlo = qs - qh.astype(np.float32) * np.float32(2.0**-17)
        qlo8 = np.clip(qlo * np.float32(2.0**20), -440, 440).astype(E4)

        # qt[p, ti, c*128 + i] = qh[q = order[ti]*128 + i, d = c*128 + p]
        def q_pack(x):
            # x: [SQ, D] -> [128p, 16t, 512(c*128+i)]
            xt = x.T.reshape(DC, P, QT_TILES, P)  # [c, p, t, i]
            xt = xt.transpose(1, 2, 0, 3)  # [p, t, c, i]
            xt = xt[:, order, :, :]  # processing order
            return np.ascontiguousarray(xt.reshape(P, QT_TILES, 512))

        qt_h = q_pack(qh)
        qt8p_h = q_pack(qlo8)

        in_maps.append(
            {
                "kh": kh_pm,
                "vh": vh_pm,
                "k8p": np.ascontiguousarray(k8p),
                "v8p": np.ascontiguousarray(v8p),
                "qt": qt_h,
                "qt8p": np.ascontiguousarray(qt8p_h),
                "mask": mask_packed,
            }
        )

    res = run_bass_kernel_spmd(
        nc, in_maps, core_ids=list(range(N_CORES)), trace=_trace
    )

    out = np.empty((B, SQ, DV), dtype=np.float32)
    inv_rows = np.empty((QT_TILES, P), dtype=np.int64)
    for ti, t in enumerate(order):
        inv_rows[ti] = perm[t * P : (t + 1) * P]
    for b in range(N_CORES):
        e_pack = np.asarray(res.results[b]["o"], dtype=np.float32)
        ob = np.zeros((SQ, DV), dtype=np.float32)
        for ti in range(QT_TILES):
            w = widths[ti]
            e = e_pack[:, offs[ti] : offs[ti] + w]
            ob[inv_rows[ti], :w] = e / e.sum(axis=-1, keepdims=True)
        out[b] = ob
    if _trace:
        kernel.last_result = res
    return out
